# revision 1
# baseline (speedup 1.0000x reference)
"""Trainium2 Bass kernel for CorrLayerDownsample.

Math (reference): hatx = fft2(xpsi); per-moment p: corr = ifft2(h1 * conj(h2)).real,
masked by masks_shift[shifted[p]], keep union_idx positions.

Device algorithm (per core):
  - 2D DFT of the needed maps by PE matmuls, keeping only u = 0..64 rows
    (real-input Hermitian symmetry; row u>=65 of the spectrum is recovered in
    stage-2 via a x2 fold), laid out as hatxT[v=128 part, u=65 free].
  - Per moment, four elementwise products t_rr/t_ii/t_ir/t_ri (DVE+GPSIMD wide
    ops over runs of moments sharing the m1 map), which feed PE directly as
    stationary operands; the complex combine happens inside the PSUM
    accumulation group (no separate combine pass):
      T[u, {re|im}] = sum_v P[v,u] * Wn[v, {yd}]       (stage 1, 4 matmuls)
      out[yd, xd]   = sum_u T_re[u,yd] Wm_re[u,xd] - T_im[u,yd] Wm_im[u,xd]
  - Mask multiply + DMA out.

Sharding: 8 cores = batch b (4) x moment parity (2). The moment pair pattern is
identical for the two parities (a2 is the innermost index-generation loop), so
a single SPMD program works: per-core inputs carry b's maps and the parity's
m2-side map subset.
"""

import sys

sys.path.insert(0, "/opt/trn_rl_repo")

import numpy as np

J, B, C, M, N = 4, 4, 8, 128, 128
UH = M // 2 + 1  # 65 kept u rows
NCORES = 8

_CACHE = {}


def _host_prep(la1, la2, shifted, union_idx, masks_shift):
    """Index analysis. Returns None if the fast-path assumptions fail."""
    P = la1.shape[0]
    if P % 2 != 0:
        return None
    m1 = la1[:, 0].astype(np.int64) * C + la1[:, 1]
    m2 = la2[:, 0].astype(np.int64) * C + la2[:, 1]
    if (m1 < 0).any() or (m1 >= J * C).any() or (m2 < 0).any() or (m2 >= J * C).any():
        return None
    xs, ys = union_idx // N, union_idx % N
    X, Y = np.unique(xs), np.unique(ys)
    NX, NY = len(X), len(Y)
    if NX * NY != len(union_idx) or NX > 64 or NY > 64:
        return None
    gx, gy = np.meshgrid(X, Y, indexing="ij")
    if not np.array_equal(np.sort(union_idx), np.sort((gx * N + gy).ravel())):
        return None
    # union_idx must be sorted x-major for the final scatter to be a transpose
    if not np.array_equal(union_idx, (gx * N + gy).ravel()):
        return None
    pe, po = np.arange(0, P, 2), np.arange(1, P, 2)
    if not np.array_equal(m1[pe], m1[po]):
        return None
    sub_e, sub_o = np.unique(m2[pe]), np.unique(m2[po])
    if len(sub_e) > 16 or len(sub_o) > 16 or len(sub_e) != len(sub_o):
        return None
    slot_e = np.searchsorted(sub_e, m2[pe])
    slot_o = np.searchsorted(sub_o, m2[po])
    if not np.array_equal(slot_e, slot_o):
        return None
    if not np.array_equal(shifted[pe], shifted[po]):
        return None
    order = np.lexsort((slot_e, m1[pe]))  # sorted row order, same for both halves
    m1_s, slot_s = m1[pe][order], slot_e[order]
    runs = []  # (m1, slot0, count)
    i = 0
    while i < len(m1_s):
        j = i
        while (
            j < len(m1_s)
            and m1_s[j] == m1_s[i]
            and slot_s[j] == slot_s[i] + (j - i)
        ):
            j += 1
        runs.append((int(m1_s[i]), int(slot_s[i]), j - i))
        i = j
    if len(runs) > 64:
        return None
    return dict(
        m1=m1, m2=m2, X=X, Y=Y, NX=NX, NY=NY, pe=pe, po=po,
        sub_e=sub_e, sub_o=sub_o, order=order, runs=runs,
        n_rows=len(order), nsub=len(sub_e),
    )


def _consts(prep):
    X, Y, NX, NY = prep["X"], prep["Y"], prep["NX"], prep["NY"]
    k = np.arange(M)
    th = 2 * np.pi * np.outer(k, k[:UH]) / M
    FmRe = np.cos(th).astype(np.float32)          # [m, k1] lhsT of T1
    FmIm = (-np.sin(th)).astype(np.float32)
    thn = 2 * np.pi * np.outer(k, k) / N
    FnRe = np.cos(thn).astype(np.float32)         # [n, k2] lhsT of T2
    FnIm = (-np.sin(thn)).astype(np.float32)
    thw = 2 * np.pi * np.outer(k, Y) / N
    WnRe = (np.cos(thw) / N).astype(np.float32)   # [128, NY]
    WnIm = (np.sin(thw) / N).astype(np.float32)
    cu = np.full(UH, 2.0, np.float32)
    cu[0] = 1.0
    if M % 2 == 0:
        cu[UH - 1] = 1.0
    thm = 2 * np.pi * np.outer(np.arange(UH), X) / M
    WmRe = (cu[:, None] * np.cos(thm) / M).astype(np.float32)      # [65, NX]
    WmImNeg = (-cu[:, None] * np.sin(thm) / M).astype(np.float32)  # [65, NX]
    # Karatsuba 3-mult complex product: with m1=h1r*h2r, m2=h1i*h2i,
    # m3=(h1r+h1i)*(h2r-h2i):  P_re = m1+m2, P_im = m3-m1+m2.
    # T = P_re^T A + P_im^T B  =  m1^T(A-B) + m2^T(A+B) + m3^T B,
    # where A = [WnRe|WnIm], B = [-WnIm|WnRe].
    WnS1 = np.concatenate([WnRe + WnIm, WnIm - WnRe], axis=1)   # A - B
    WnS2 = np.concatenate([WnRe - WnIm, WnIm + WnRe], axis=1)   # A + B
    WnS3 = np.concatenate([-WnIm, WnRe], axis=1)                # B
    ident = np.eye(M, dtype=np.float32)
    return dict(
        FmRe=FmRe, FmIm=FmIm, FnRe=FnRe, FnIm=FnIm, FnImNeg=(-FnIm).copy(),
        WnS1=WnS1, WnS2=WnS2, WnS3=WnS3, WmRe=WmRe, WmImNeg=WmImNeg, ident=ident,
    )


def _build_program(prep, repeat=1):
    import concourse.bacc as bacc
    import concourse.mybir as mybir
    import concourse.tile as tile

    f32 = mybir.dt.float32
    NX, NY = prep["NX"], prep["NY"]
    n_rows, nsub = prep["n_rows"], prep["nsub"]
    runs = prep["runs"]
    nmaps = J * C + nsub  # 32 m1-side + nsub m2-side maps
    W2 = 2 * NY           # stacked stage-1 rhs width

    nc = bacc.Bacc("TRN2", target_bir_lowering=False, debug=False,
                   num_devices=NCORES)

    def din(name, shape):
        return nc.dram_tensor(name, list(shape), f32, kind="ExternalInput").ap()

    xmaps = din("xmaps", (nmaps, M, N))
    FmRe, FmIm = din("FmRe", (M, UH)), din("FmIm", (M, UH))
    FnRe, FnIm = din("FnRe", (M, M)), din("FnIm", (M, M))
    FnImNeg = din("FnImNeg", (M, M))
    WnS1, WnS2, WnS3 = din("WnS1", (M, W2)), din("WnS2", (M, W2)), din("WnS3", (M, W2))
    WmRe, WmImNeg = din("WmRe", (UH, NX)), din("WmImNeg", (UH, NX))
    ident = din("ident", (M, M))
    maskv = din("maskv", (NY, n_rows * NX))
    out = nc.dram_tensor("out", [n_rows, NY, NX], f32, kind="ExternalOutput").ap()

    GT = 512 // W2            # stage-1 T tiles per PSUM bank (15 for NY=17)
    GO = 512 // NX            # stage-2 outs per PSUM bank  (30 for NX=17)
    GO = min(GO, 30)

    with tile.TileContext(nc) as tc:
        with tc.tile_pool(name="const", bufs=1) as cpool:
            c_FmRe = cpool.tile([M, UH], f32)
            c_FmIm = cpool.tile([M, UH], f32)
            c_FnRe = cpool.tile([M, M], f32)
            c_FnIm = cpool.tile([M, M], f32)
            c_FnImNeg = cpool.tile([M, M], f32)
            c_Wn1 = cpool.tile([M, W2], f32)
            c_Wn2 = cpool.tile([M, W2], f32)
            c_Wn3 = cpool.tile([M, W2], f32)
            c_WmRe = cpool.tile([UH, NX], f32)
            c_WmImNeg = cpool.tile([UH, NX], f32)
            c_id = cpool.tile([M, M], f32)
            c_mask = cpool.tile([NY, n_rows * NX], f32)
            for t, s in [
                (c_FmRe, FmRe), (c_FmIm, FmIm), (c_FnRe, FnRe), (c_FnIm, FnIm),
                (c_FnImNeg, FnImNeg), (c_Wn1, WnS1), (c_Wn2, WnS2), (c_Wn3, WnS3),
                (c_WmRe, WmRe), (c_WmImNeg, WmImNeg), (c_id, ident), (c_mask, maskv),
            ]:
                nc.sync.dma_start(t[:], s[:])

            hat_ctx = tc.tile_pool(name="hatx", bufs=1)
            hat_pool = hat_ctx.__enter__()
            hat_re = hat_pool.tile([M, nmaps * UH], f32)
            hat_im = hat_pool.tile([M, nmaps * UH], f32)

            # ---------------- FFT phase ----------------
            # m2-side maps first so the main loop's first runs unblock early;
            # copies ride GPSIMD (DVE is idle here, ACT stays free for later).
            fft_order = list(range(J * C, nmaps)) + list(range(J * C))
            with tc.tile_pool(name="fftsb", bufs=4) as fsb, \
                 tc.tile_pool(name="fftps", bufs=2, space="PSUM") as fps, \
                 tc.tile_pool(name="fftps2", bufs=2, space="PSUM") as fps2, \
                 tc.tile_pool(name="fftps3", bufs=2, space="PSUM") as fps3:
                xbig = fsb.tile([M, nmaps * N], f32, tag="xbig")
                nc.sync.dma_start(
                    xbig[:].rearrange("p (z n) -> p z n", z=nmaps),
                    xmaps[:].transpose([1, 0, 2]))
                for _rep in range(repeat):
                  for z in fft_order:
                    xt = xbig[:, z * N:(z + 1) * N]
                    # T1: A[k1, n] = Fm^T x   (complex planes side by side)
                    pA = fps.tile([UH, 2 * N], f32, tag="pA")
                    nc.tensor.matmul(pA[:, 0:N], c_FmRe[:], xt, start=True, stop=True)
                    nc.tensor.matmul(pA[:, N:2 * N], c_FmIm[:], xt, start=True, stop=True)
                    sA = fsb.tile([UH, 2 * N], f32, tag="sA")
                    nc.vector.tensor_copy(sA[:], pA[:])
                    # transpose both planes -> AT [n, k1]
                    pT = fps2.tile([M, 2 * UH], f32, tag="pT")
                    nc.tensor.transpose(pT[:, 0:UH], sA[:, 0:N], c_id[0:UH, 0:UH])
                    nc.tensor.transpose(pT[:, UH:2 * UH], sA[:, N:2 * N], c_id[0:UH, 0:UH])
                    sT = fsb.tile([M, 2 * UH], f32, tag="sT")
                    nc.vector.tensor_copy(sT[:], pT[:])
                    # T2: B[k2, k1] = Fn^T AT (complex)
                    pB = fps3.tile([M, 2 * UH], f32, tag="pB")
                    nc.tensor.matmul(pB[:, 0:UH], c_FnRe[:], sT[:, 0:UH], start=True, stop=False)
                    nc.tensor.matmul(pB[:, 0:UH], c_FnImNeg[:], sT[:, UH:2 * UH], start=False, stop=True)
                    nc.tensor.matmul(pB[:, UH:2 * UH], c_FnRe[:], sT[:, UH:2 * UH], start=True, stop=False)
                    nc.tensor.matmul(pB[:, UH:2 * UH], c_FnIm[:], sT[:, 0:UH], start=False, stop=True)
                    nc.vector.tensor_copy(hat_re[:, z * UH:(z + 1) * UH], pB[:, 0:UH])
                    nc.vector.tensor_copy(hat_im[:, z * UH:(z + 1) * UH], pB[:, UH:2 * UH])

            # ---------------- main loop ----------------
            h2_re = hat_re[:, J * C * UH:]  # m2-side maps
            h2_im = hat_im[:, J * C * UH:]
            # Karatsuba sum planes: hs1 = h1r+h1i (m1 side), hs2 = h2r-h2i
            hs1 = hat_pool.tile([M, J * C * UH], f32)
            hs2 = hat_pool.tile([M, nsub * UH], f32)
            nc.vector.tensor_add(hs1[:], hat_re[:, :J * C * UH], hat_im[:, :J * C * UH])
            nc.vector.tensor_sub(hs2[:], h2_re, h2_im)

            with tc.tile_pool(name="tt", bufs=5) as tpool, \
                 tc.tile_pool(name="tsb", bufs=2) as tsbp, \
                 tc.tile_pool(name="stg", bufs=2) as stgp, \
                 tc.tile_pool(name="psT", bufs=3, space="PSUM") as psT, \
                 tc.tile_pool(name="psO", bufs=3, space="PSUM") as psO:

                # flat list of per-row t-tile APs, filled run by run
                row_t = [None] * (n_rows * repeat)
                r0 = 0
                for (a, s0, R) in runs * repeat:
                    t_m1 = tpool.tile([M, R * UH], f32, tag="t_m1")
                    t_m2 = tpool.tile([M, R * UH], f32, tag="t_m2")
                    t_m3 = tpool.tile([M, R * UH], f32, tag="t_m3")
                    a_re = hat_re[:, a * UH:(a + 1) * UH].unsqueeze(1).broadcast_to([M, R, UH])
                    a_im = hat_im[:, a * UH:(a + 1) * UH].unsqueeze(1).broadcast_to([M, R, UH])
                    a_s = hs1[:, a * UH:(a + 1) * UH].unsqueeze(1).broadcast_to([M, R, UH])
                    b_re = h2_re[:, s0 * UH:(s0 + R) * UH].rearrange("p (r u) -> p r u", r=R)
                    b_im = h2_im[:, s0 * UH:(s0 + R) * UH].rearrange("p (r u) -> p r u", r=R)
                    b_s = hs2[:, s0 * UH:(s0 + R) * UH].rearrange("p (r u) -> p r u", r=R)
                    v_m1 = t_m1[:].rearrange("p (r u) -> p r u", r=R)
                    v_m2 = t_m2[:].rearrange("p (r u) -> p r u", r=R)
                    v_m3 = t_m3[:].rearrange("p (r u) -> p r u", r=R)
                    nc.vector.tensor_mul(v_m1, a_re, b_re)
                    nc.vector.tensor_mul(v_m2, a_im, b_im)
                    nc.vector.tensor_mul(v_m3, a_s, b_s)
                    for i in range(R):
                        row_t[r0 + i] = (
                            t_m1[:, i * UH:(i + 1) * UH],
                            t_m2[:, i * UH:(i + 1) * UH],
                            t_m3[:, i * UH:(i + 1) * UH],
                        )
                    r0 += R

                # stage-1 groups of GT rows -> one PSUM bank + one batched copy
                Tsb = [None] * (n_rows * repeat)
                for g0 in range(0, n_rows * repeat, GT):
                    g = min(GT, n_rows * repeat - g0)
                    pT1 = psT.tile([UH, g * W2], f32, tag="pT1")
                    for i in range(g):
                        tm1, tm2, tm3 = row_t[g0 + i]
                        o = pT1[:, i * W2:(i + 1) * W2]
                        nc.tensor.matmul(o, tm1, c_Wn1[:], start=True, stop=False)
                        nc.tensor.matmul(o, tm2, c_Wn2[:], start=False, stop=False)
                        nc.tensor.matmul(o, tm3, c_Wn3[:], start=False, stop=True)
                    sT1 = tsbp.tile([UH, g * W2], f32, tag="sT1")
                    nc.scalar.copy(sT1[:], pT1[:])
                    for i in range(g):
                        Tsb[g0 + i] = sT1[:, i * W2:(i + 1) * W2]

                # stage-2 + mask + out DMA, groups of GO rows
                g0 = 0
                while g0 < n_rows * repeat:
                    g0m = g0 % n_rows
                    g = min(GO, n_rows * repeat - g0, n_rows - g0m)
                    pO = psO.tile([NY, g * NX], f32, tag="pO")
                    for i in range(g):
                        T = Tsb[g0 + i]
                        o = pO[:, i * NX:(i + 1) * NX]
                        nc.tensor.matmul(o, T[:, 0:NY], c_WmRe[:], start=True, stop=False)
                        nc.tensor.matmul(o, T[:, NY:2 * NY], c_WmImNeg[:], start=False, stop=True)
                    stg = stgp.tile([NY, g * NX], f32, tag="stg")
                    nc.vector.tensor_mul(stg[:], pO[:], c_mask[:, g0m * NX:(g0m + g) * NX])
                    nc.sync.dma_start(
                        out[g0m:g0m + g].transpose([1, 0, 2]),
                        stg[:].rearrange("p (r x) -> p r x", r=g),
                    )
                    g0 += g
            hat_ctx.__exit__(None, None, None)

    nc.compile()
    return nc


def _fallback(xpsi, masks_shift, la1, la2, shifted, union_idx):
    hatx = np.fft.fft2(xpsi.astype(np.float64))
    h1 = hatx[la1[:, 0], :, la1[:, 1]]
    h2 = hatx[la2[:, 0], :, la2[:, 1]]
    corr = np.fft.ifft2(h1 * np.conj(h2)).real
    masked = corr * masks_shift[shifted][:, None]
    Pm, Bb, Mm, Nn = masked.shape
    return masked.reshape(Pm, Bb, Mm * Nn)[:, :, union_idx].astype(np.float32)


def kernel(**inputs):
    xpsi = np.ascontiguousarray(np.asarray(inputs["xpsi"], dtype=np.float32))
    masks_shift = np.asarray(inputs["masks_shift"], dtype=np.float32)
    la1 = np.asarray(inputs["la1"], dtype=np.int64)
    la2 = np.asarray(inputs["la2"], dtype=np.int64)
    shifted = np.asarray(inputs["shifted"], dtype=np.int64)
    union_idx = np.asarray(inputs["union_idx"], dtype=np.int64)

    if xpsi.shape != (J, B, C, M, N) or (shifted < 0).any() or \
            (shifted >= masks_shift.shape[0]).any():
        return _fallback(xpsi, masks_shift, la1, la2, shifted, union_idx)
    prep = _host_prep(la1, la2, shifted, union_idx, masks_shift)
    if prep is None:
        return _fallback(xpsi, masks_shift, la1, la2, shifted, union_idx)

    try:
        return _run_device(xpsi, masks_shift, la1, shifted, union_idx, prep)
    except Exception:
        return _fallback(xpsi, masks_shift, la1, la2, shifted, union_idx)


def _run_device(xpsi, masks_shift, la1, shifted, union_idx, prep):
    key = (prep["NX"], prep["NY"], prep["n_rows"], tuple(prep["runs"]))
    if key not in _CACHE:
        _CACHE[key] = _build_program(prep)
    nc = _CACHE[key]

    cst = _consts(prep)
    X, Y, NX, NY = prep["X"], prep["Y"], prep["NX"], prep["NY"]
    pe, order = prep["pe"], prep["order"]
    n_rows = prep["n_rows"]

    # per-row mask values: maskv[y, row*NX + x] = masks[shifted[p], X[x], Y[y]]
    p_sorted_even = pe[order]                       # original even p per row
    mk = masks_shift[shifted[p_sorted_even]]        # [n_rows, 128, 128]
    mv = mk[:, X[:, None], Y[None, :]]              # [n_rows, NX, NY]
    maskv = np.ascontiguousarray(
        mv.transpose(2, 0, 1).reshape(NY, n_rows * NX))

    xflat = xpsi.transpose(0, 2, 1, 3, 4).reshape(J * C, B, M, N)
    in_maps = []
    for core in range(NCORES):
        b, parity = divmod(core, 2)
        sub = prep["sub_e"] if parity == 0 else prep["sub_o"]
        xm = np.concatenate(
            [xflat[:, b], xflat[sub, b]], axis=0).astype(np.float32)
        in_maps.append({
            "xmaps": np.ascontiguousarray(xm),
            "FmRe": cst["FmRe"], "FmIm": cst["FmIm"],
            "FnRe": cst["FnRe"], "FnIm": cst["FnIm"], "FnImNeg": cst["FnImNeg"],
            "WnS1": cst["WnS1"], "WnS2": cst["WnS2"], "WnS3": cst["WnS3"],
            "WmRe": cst["WmRe"], "WmImNeg": cst["WmImNeg"],
            "ident": cst["ident"], "maskv": maskv,
        })

    from concourse.bass_utils import run_bass_kernel_spmd
    res = run_bass_kernel_spmd(nc, in_maps, list(range(NCORES)))

    P = la1.shape[0]
    out = np.empty((P, B, len(union_idx)), np.float32)
    inv = np.empty(n_rows, np.int64)
    inv[order] = np.arange(n_rows)                  # row of sorted order for pe[k]
    for core in range(NCORES):
        b, parity = divmod(core, 2)
        dev = res.results[core]["out"]              # [n_rows, NY, NX]
        flat = dev.transpose(0, 2, 1).reshape(n_rows, NX * NY)  # x-major
        p_idx = prep["pe"] if parity == 0 else prep["po"]
        out[p_idx, b, :] = flat[inv]
    return out


if __name__ == "__main__":
    import importlib
    ref = importlib.import_module("reference")
    import jax
    cpu = jax.devices("cpu")[0]
    with jax.default_device(cpu):
        raw = ref.setup_inputs()
        ins = {k: np.asarray(v) for k, v in raw.items()}
        exp = np.asarray(ref.reference(**{k: jax.device_put(v, cpu) for k, v in raw.items()}))
    got = kernel(**ins)
    d = np.linalg.norm(got - exp) / np.linalg.norm(exp)
    print("rel:", d, "maxabs:", np.abs(got - exp).max())



# revision 2
# speedup vs baseline: 1.6606x; 1.6606x over previous
"""Trainium2 Bass kernel for CorrLayerDownsample (optimized v4).

Math: hatx = fft2(xpsi); per pair p: corr = ifft2(h1 * conj(h2)).real, masked by
masks_shift[shifted[p]], keep union_idx positions (a 17x17 circular patch).

Pair structure (verified at runtime, else fallback): the 640 pairs are exactly
{(m1, m2): 0<=m1,m2<32, m2//8 >= m1//8} over the 32 (scale,channel) maps, and
shifted depends only on m2//8.  Sharding: 8 cores = batch b (4) x m2-parity (2).
Per-core map order [parity maps sorted (16) | other-parity maps sorted (16)]
makes one static SPMD program serve both parities: the b-side maps are always
slots 0..15, and a-side slot k pairs with b-slots [4*j1, 16), j1=(k%16)//4.

Per-core pipeline (instruction-count-minimal):
  T1': AT[n, {re|im}u] = x^T Fm-stack per map -- stationary = x (f32r,
       self-loading), moving = [FmRe|FmIm]; no PE transposes, no staging.
  T2:  hat[v, u] = Fn^T AT (fp32 self-loading matmuls, 7 maps each) -> bf16.
  Products: 3 Karatsuba planes per 4-run merged group (24 wide DVE/Pool ops,
       bf16 2x on DVE).
  Stage 1: T^T[yd', (r u)] = Wn_k^T t_k, batched <=7 rows/matmul (bf16).
  Transposes: [34,65] -> [65,34] per row (fp32 self-loading transpose).
  Stage 2: out[xd, (r yd)] = Wm^T T, batched 15 rows / 2 matmuls (fp32).
  Mask multiply + DMA out.
"""

import sys

sys.path.insert(0, "/opt/trn_rl_repo")

import numpy as np

J, B, C, M, N = 4, 4, 8, 128, 128
UH = M // 2 + 1  # 65 kept u rows
U2 = 2 * UH      # 130: re|im stacked
NMAPS = J * C    # 32
NCORES = 8
NX = NY = 17
GT = 15          # stage-2 rows per PSUM bank (15*34=510 <= 512)
GR = 7           # stage-1 rows per matmul (7*65=455 <= 512)

_CACHE = {}


def _row_table():
    rows = []
    for sa in range(2 * 16):
        j1 = (sa % 16) // 4
        for sb in range(4 * j1, 16):
            rows.append((sa, sb))
    return rows


def _host_prep(la1, la2, shifted, union_idx, masks_shift):
    """Verify the pair/mask/union structure. Returns None on mismatch."""
    P = la1.shape[0]
    if P != 640 or masks_shift.shape != (J + 1, M, N):
        return None
    m1 = la1[:, 0].astype(np.int64) * C + la1[:, 1]
    m2 = la2[:, 0].astype(np.int64) * C + la2[:, 1]
    if (m1 < 0).any() or (m1 >= NMAPS).any() or (m2 < 0).any() or (m2 >= NMAPS).any():
        return None
    if (shifted < 0).any() or (shifted >= J + 1).any():
        return None
    pairidx = {}
    for i in range(P):
        key = (int(m1[i]), int(m2[i]))
        if key in pairidx:
            return None
        pairidx[key] = i
    want = {(a, b) for a in range(NMAPS) for b in range(NMAPS) if b // 8 >= a // 8}
    if set(pairidx) != want:
        return None
    # union grid: 17x17, x-major sorted
    xs, ys = union_idx // N, union_idx % N
    X, Y = np.unique(xs), np.unique(ys)
    if len(X) != NX or len(Y) != NY:
        return None
    gx, gy = np.meshgrid(X, Y, indexing="ij")
    if not np.array_equal(union_idx, (gx * N + gy).ravel()):
        return None
    rows = _row_table()
    if len(rows) != 320:
        return None
    # per-parity row -> original pair index; mask must agree across parity
    ridx = np.zeros((2, len(rows)), np.int64)
    for p in (0, 1):
        for r, (sa, sb) in enumerate(rows):
            k = sa % 16
            mm1 = 2 * k + (p if sa < 16 else 1 - p)
            mm2 = 2 * sb + p
            ridx[p, r] = pairidx[(mm1, mm2)]
    if not np.array_equal(shifted[ridx[0]], shifted[ridx[1]]):
        return None
    return dict(X=X, Y=Y, rows=rows, ridx=ridx, n_rows=len(rows))


def _consts(prep, masks_shift, shifted):
    X, Y = prep["X"], prep["Y"]
    k = np.arange(M)
    th = 2 * np.pi * np.outer(k, k[:UH]) / M
    FmRe = np.cos(th).astype(np.float32)          # [m, k1]
    FmIm = (-np.sin(th)).astype(np.float32)
    FmS = np.concatenate([FmRe, FmIm], axis=1)    # [m, 130] moving of T1'
    thn = 2 * np.pi * np.outer(k, k) / N
    FnRe = np.cos(thn).astype(np.float32)         # [n, k2] lhsT of T2
    FnIm = (-np.sin(thn)).astype(np.float32)
    thw = 2 * np.pi * np.outer(k, Y) / N
    WnRe = (np.cos(thw) / N).astype(np.float32)   # [128, NY]
    WnIm = (np.sin(thw) / N).astype(np.float32)
    cu = np.full(UH, 2.0, np.float32)
    cu[0] = 1.0
    cu[UH - 1] = 1.0
    thm = 2 * np.pi * np.outer(np.arange(UH), X) / M
    WmRe = (cu[:, None] * np.cos(thm) / M).astype(np.float32)      # [65, NX]
    WmImNeg = (-cu[:, None] * np.sin(thm) / M).astype(np.float32)  # [65, NX]
    # Karatsuba 3-mult complex product: m1=h1r*h2r, m2=h1i*h2i,
    # m3=(h1r+h1i)*(h2r-h2i):  P_re = m1+m2, P_im = m3-m1+m2.
    # T = P_re^T A + P_im^T B  =  m1^T(A-B) + m2^T(A+B) + m3^T B,
    # where A = [WnRe|WnIm], B = [-WnIm|WnRe].
    WnS1 = np.concatenate([WnRe + WnIm, WnIm - WnRe], axis=1)   # A - B
    WnS2 = np.concatenate([WnRe - WnIm, WnIm + WnRe], axis=1)   # A + B
    WnS3 = np.concatenate([-WnIm, WnRe], axis=1)                # B
    ident = np.eye(M, dtype=np.float32)
    # maskv[x, r*NY + y] = masks[shifted(row r)][X[x], Y[y]]  (parity-0 rows)
    n_rows = prep["n_rows"]
    mk = masks_shift[shifted[prep["ridx"][0]]]      # [n_rows, 128, 128]
    mv = mk[:, X[:, None], Y[None, :]]              # [n_rows, NX, NY]
    maskv = np.ascontiguousarray(
        mv.transpose(1, 0, 2).reshape(NX, n_rows * NY))
    import concourse.mybir as mybir
    bf16 = mybir.dt.np(mybir.dt.bfloat16)
    # pack f32 consts [128, CW]: FnRe|FnIm|FnImNeg|ident|Wm pair|mask folded
    nch = (n_rows + 119) // 120
    CW = M + nch * 120 * NY
    cf32 = np.zeros((M, CW), np.float32)
    cf32[:, 0:M] = ident
    mh = M
    for ci in range(nch):
        r0, r1 = ci * 120, min((ci + 1) * 120, n_rows)
        cf32[0:NX, mh + ci * 120 * NY: mh + ci * 120 * NY + (r1 - r0) * NY] = \
            maskv[:, r0 * NY:r1 * NY]
    cbf = np.zeros((M, 3 * 2 * NY + 2 * NX + 3 * M), np.float32)
    cbf[:, 0:6 * NY] = np.concatenate([WnS1, WnS2, WnS3], axis=1)
    cbf[0:UH, 6 * NY:6 * NY + NX] = WmRe
    cbf[0:UH, 6 * NY + NX:6 * NY + 2 * NX] = WmImNeg
    cb0 = 6 * NY + 2 * NX
    cbf[:, cb0:cb0 + M] = FnRe
    cbf[:, cb0 + M:cb0 + 2 * M] = FnIm
    cbf[:, cb0 + 2 * M:cb0 + 3 * M] = -FnIm
    cbf = cbf.astype(bf16)
    return dict(FmS=FmS, cf32=cf32, cbf=cbf)


def _build_program(prep, repeat=1):
    import concourse.bacc as bacc
    import concourse.mybir as mybir
    import concourse.tile as tile

    f32 = mybir.dt.float32
    f32r = mybir.dt.float32r
    bf16 = mybir.dt.bfloat16
    n_rows = prep["n_rows"]
    W2 = 2 * NY  # 34

    nc = bacc.Bacc("TRN2", target_bir_lowering=False, debug=False,
                   num_devices=NCORES)

    def din(name, shape, dt=f32):
        return nc.dram_tensor(name, list(shape), dt, kind="ExternalInput").ap()

    xmapsT = din("xmapsT", (M, NMAPS * N), f32r)    # pre-transposed on host
    FmS = din("FmS", (M, U2), f32r)
    # cf32: [FnRe|FnIm|FnImNeg|ident|WmRe+WmImNeg+maskv padded to 128 rows]
    CW = M + ((n_rows + 119) // 120) * 120 * NY
    cf32 = din("cf32", (M, CW))
    cbf = din("cbf", (M, 3 * W2 + 2 * NX + 3 * M), bf16)
    out = nc.dram_tensor("out", [n_rows, NX, NY], f32, kind="ExternalOutput").ap()

    with tile.TileContext(nc) as tc:
        with tc.tile_pool(name="const", bufs=1) as cpool:
            c_FmS = cpool.tile([M, U2], f32r)
            c_f32 = cpool.tile([M, CW], f32)
            c_bf = cpool.tile([M, 3 * W2 + 2 * NX + 3 * M], bf16)
            nc.sync.dma_start(c_FmS[:], FmS[:])
            nc.sync.dma_start(c_f32[:], cf32[:])
            nc.sync.dma_start(c_bf[:], cbf[:])
            c_id = c_f32[:, 0:M]
            # maskv folded: column chunk ci holds rows [120*ci, ...)
            mh = M
            c_Wn1 = c_bf[:, 0:W2]
            c_Wn2 = c_bf[:, W2:2 * W2]
            c_Wn3 = c_bf[:, 2 * W2:3 * W2]
            c_WmRe = c_bf[0:UH, 3 * W2:3 * W2 + NX]
            c_WmImNeg = c_bf[0:UH, 3 * W2 + NX:3 * W2 + 2 * NX]
            cb0 = 3 * W2 + 2 * NX
            c_FnRe = c_bf[:, cb0:cb0 + M]
            c_FnIm = c_bf[:, cb0 + M:cb0 + 2 * M]
            c_FnImNeg = c_bf[:, cb0 + 2 * M:cb0 + 3 * M]

            stg_all = cpool.tile([NX, n_rows * NY], f32)
            sT_all = cpool.tile([UH, n_rows * W2], bf16)
            MCH = 120 * NY  # mask fold chunk width (120 rows per chunk)

            def c_mask_view(r0, g):
                ci, rr = divmod(r0, 120)
                assert rr + g <= 120
                base = mh + ci * MCH
                return c_f32[0:NX, base + rr * NY: base + (rr + g) * NY]

            xbig = cpool.tile([M, NMAPS * N], f32r)      # [p, z*128+n]
            AT = cpool.tile([M, NMAPS * U2], bf16)       # [n, z*130 + {re|im}u]
            hat_re = cpool.tile([M, NMAPS * UH], bf16)   # [v, z*65+u]
            hat_im = cpool.tile([M, NMAPS * UH], bf16)
            hs1 = cpool.tile([M, NMAPS * UH], bf16)      # hat_re + hat_im
            hs2 = cpool.tile([M, 16 * UH], bf16)         # b-side: re - im

            # staged input DMA: 4-map chunks to let T1' start early
            for g in range(8):
                nc.sync.dma_start(xbig[:, g * 512:(g + 1) * 512],
                                  xmapsT[:, g * 512:(g + 1) * 512])

            for _rep in range(repeat):
                # ---------------- FFT phase ----------------
                # T1': AT_z = x_z^T @ [FmRe|FmIm]; stationary = x (f32r
                # self-loading), moving = FmS. 3 maps per PSUM bank.
                with tc.tile_pool(name="fpA", bufs=3, space="PSUM") as fpA:
                    for g0 in range(0, NMAPS, 3):
                        gn = min(3, NMAPS - g0)
                        pa = fpA.tile([M, 3 * U2], f32, tag="pa")
                        for j in range(gn):
                            z = g0 + j
                            nc.tensor.matmul(
                                pa[:, j * U2:(j + 1) * U2],
                                xbig[:, z * N:(z + 1) * N], c_FmS[:],
                                start=True, stop=True)
                        nc.scalar.copy(AT[:, g0 * U2:(g0 + gn) * U2],
                                       pa[:, 0:gn * U2])

                # T2: hat = Fn^T AT (fp32 self-loading), 7 maps per matmul
                with tc.tile_pool(name="fph", bufs=2, space="PSUM") as fph, \
                     tc.tile_pool(name="fph2", bufs=2, space="PSUM") as fph2:
                    zgroups = [(0, 7), (7, 14), (14, 21), (21, 28), (28, 32)]
                    for z0, z1 in zgroups:
                        g = z1 - z0
                        zsl = slice(z0 * UH, z1 * UH)
                        pre = fph.tile([M, 7 * UH], f32, tag="pre")
                        pim = fph2.tile([M, 7 * UH], f32, tag="pim")
                        atv = AT[:, z0 * U2:z1 * U2].rearrange(
                            "p (z c) -> p z c", c=U2)
                        are = atv[:, :, 0:UH]
                        aim = atv[:, :, UH:U2]
                        w = g * UH
                        nc.tensor.matmul(pre[:, 0:w], c_FnRe, are,
                                         start=True, stop=False)
                        nc.tensor.matmul(pre[:, 0:w], c_FnImNeg, aim,
                                         start=False, stop=True)
                        nc.tensor.matmul(pim[:, 0:w], c_FnRe, aim,
                                         start=True, stop=False)
                        nc.tensor.matmul(pim[:, 0:w], c_FnIm, are,
                                         start=False, stop=True)
                        nc.vector.tensor_copy(hat_re[:, zsl], pre[:, 0:w])
                        nc.vector.tensor_copy(hat_im[:, zsl], pim[:, 0:w])
                        # Karatsuba sum planes as soon as the group lands
                        nc.vector.tensor_add(hs1[:, zsl], hat_re[:, zsl],
                                             hat_im[:, zsl])
                        if z0 < 16:
                            b1 = min(z1, 16)
                            bsl = slice(z0 * UH, b1 * UH)
                            nc.vector.tensor_sub(hs2[:, bsl], hat_re[:, bsl],
                                                 hat_im[:, bsl])

                # ---------------- main loop ----------------
                with tc.tile_pool(name="tt", bufs=2) as tpool, \
                     tc.tile_pool(name="tsT", bufs=8) as tsTT, \
                     tc.tile_pool(name="psG", bufs=3, space="PSUM") as psG, \
                     tc.tile_pool(name="psT", bufs=3, space="PSUM") as psT, \
                     tc.tile_pool(name="psO", bufs=2, space="PSUM") as psO:

                    # products: 4 runs (same j1, same b-range) merged per op
                    mg_tiles = []        # (t1, t2, t3, nrows_in_tile)
                    for h in range(2):           # parity halves of sa
                        for j1 in range(4):
                            sa0 = 16 * h + 4 * j1
                            s0, R = 4 * j1, 16 - 4 * j1
                            nr = 4 * R
                            t_m1 = tpool.tile([M, 4 * 16 * UH], bf16, tag="t_m1")
                            t_m2 = tpool.tile([M, 4 * 16 * UH], bf16, tag="t_m2")
                            t_m3 = tpool.tile([M, 4 * 16 * UH], bf16, tag="t_m3")
                            asl = slice(sa0 * UH, (sa0 + 4) * UH)
                            bsl = slice(s0 * UH, (s0 + R) * UH)
                            a_re = hat_re[:, asl].rearrange(
                                "p (s u) -> p s u", s=4).unsqueeze(2) \
                                .broadcast_to([M, 4, R, UH])
                            a_im = hat_im[:, asl].rearrange(
                                "p (s u) -> p s u", s=4).unsqueeze(2) \
                                .broadcast_to([M, 4, R, UH])
                            a_s = hs1[:, asl].rearrange(
                                "p (s u) -> p s u", s=4).unsqueeze(2) \
                                .broadcast_to([M, 4, R, UH])
                            b_re = hat_re[:, bsl].rearrange(
                                "p (r u) -> p r u", r=R).unsqueeze(1) \
                                .broadcast_to([M, 4, R, UH])
                            b_im = hat_im[:, bsl].rearrange(
                                "p (r u) -> p r u", r=R).unsqueeze(1) \
                                .broadcast_to([M, 4, R, UH])
                            b_s = hs2[:, bsl].rearrange(
                                "p (r u) -> p r u", r=R).unsqueeze(1) \
                                .broadcast_to([M, 4, R, UH])
                            v1 = t_m1[:, 0:nr * UH].rearrange(
                                "p (s r u) -> p s r u", s=4, r=R)
                            v2 = t_m2[:, 0:nr * UH].rearrange(
                                "p (s r u) -> p s r u", s=4, r=R)
                            v3 = t_m3[:, 0:nr * UH].rearrange(
                                "p (s r u) -> p s r u", s=4, r=R)
                            nc.vector.tensor_mul(v1, a_re, b_re)
                            if j1 < 2:
                                nc.gpsimd.tensor_mul(v2, a_im, b_im)
                            else:
                                nc.vector.tensor_mul(v2, a_im, b_im)
                            nc.vector.tensor_mul(v3, a_s, b_s)
                            mg_tiles.append((t_m1, t_m2, t_m3, nr))

                    # stage 1 batched (bf16) + per-row fp32 transposes +
                    # stage 2 batched (fp32) per GT rows
                    sTT_rows = []        # per global row: (sbuf tile, offset)
                    for (t_m1, t_m2, t_m3, nr) in mg_tiles:
                        for i0 in range(0, nr, GR):
                            g = min(GR, nr - i0)
                            isl = slice(i0 * UH, (i0 + g) * UH)
                            pG = psG.tile([W2, GR * UH], f32, tag="pG")
                            o = pG[:, 0:g * UH]
                            nc.tensor.matmul(o, c_Wn1, t_m1[:, isl],
                                             start=True, stop=False)
                            nc.tensor.matmul(o, c_Wn2, t_m2[:, isl],
                                             start=False, stop=False)
                            nc.tensor.matmul(o, c_Wn3, t_m3[:, isl],
                                             start=False, stop=True)
                            sg = tsTT.tile([W2, GR * UH], f32, tag="sg")
                            nc.scalar.copy(sg[:, 0:g * UH], o)
                            for i in range(g):
                                sTT_rows.append((sg, i * UH))

                    g0 = 0
                    while g0 < n_rows:
                        g = min(GT, n_rows - g0)
                        pT1 = psT.tile([UH, GT * W2], f32, tag="pT1")
                        for i in range(g):
                            sg, off = sTT_rows[g0 + i]
                            nc.tensor.transpose(
                                pT1[:, i * W2:(i + 1) * W2],
                                sg[:, off:off + UH], c_id[0:W2, 0:W2])
                        nc.scalar.copy(sT_all[:, g0 * W2:(g0 + g) * W2],
                                       pT1[:, 0:g * W2])
                        g0 += g
                    # stage 2: out[xd, (r yd)] = Wm^T T (bf16, 30 rows/pair)
                    G2 = 30
                    g0 = 0
                    while g0 < n_rows:
                        g = min(G2, n_rows - g0)
                        tv = sT_all[:, g0 * W2:(g0 + g) * W2].rearrange(
                            "p (r c) -> p r c", c=W2)
                        t_re = tv[:, 0:g, 0:NY]
                        t_im = tv[:, 0:g, NY:W2]
                        pO = psO.tile([NX, G2 * NY], f32, tag="pO")
                        nc.tensor.matmul(pO[:, 0:g * NY], c_WmRe, t_re,
                                         start=True, stop=False)
                        nc.tensor.matmul(pO[:, 0:g * NY], c_WmImNeg, t_im,
                                         start=False, stop=True)
                        msl = c_mask_view(g0, g)
                        nc.vector.tensor_mul(
                            stg_all[:, g0 * NY:(g0 + g) * NY],
                            pO[:, 0:g * NY], msl)
                        g0 += g
                    nc.sync.dma_start(
                        out[:].transpose([1, 0, 2]),
                        stg_all[:].rearrange("p (r y) -> p r y", r=n_rows),
                    )

    nc.compile()
    return nc


def _fallback(xpsi, masks_shift, la1, la2, shifted, union_idx):
    hatx = np.fft.fft2(xpsi.astype(np.float64))
    h1 = hatx[la1[:, 0], :, la1[:, 1]]
    h2 = hatx[la2[:, 0], :, la2[:, 1]]
    corr = np.fft.ifft2(h1 * np.conj(h2)).real
    masked = corr * masks_shift[shifted][:, None]
    Pm, Bb, Mm, Nn = masked.shape
    return masked.reshape(Pm, Bb, Mm * Nn)[:, :, union_idx].astype(np.float32)


def _make_in_maps(xpsi, prep, cst):
    xflat = xpsi.transpose(0, 2, 1, 3, 4).reshape(NMAPS, B, M, N)
    in_maps = []
    for core in range(NCORES):
        b, p = divmod(core, 2)
        ids = list(range(p, NMAPS, 2)) + list(range(1 - p, NMAPS, 2))
        xm = xflat[ids, b]                            # [32, 128, 128]
        xmT = np.ascontiguousarray(
            xm.transpose(1, 0, 2).reshape(M, NMAPS * N)).astype(np.float32)
        in_maps.append({"xmapsT": xmT, "FmS": cst["FmS"],
                        "cf32": cst["cf32"], "cbf": cst["cbf"]})
    return in_maps


def kernel(**inputs):
    xpsi = np.ascontiguousarray(np.asarray(inputs["xpsi"], dtype=np.float32))
    masks_shift = np.asarray(inputs["masks_shift"], dtype=np.float32)
    la1 = np.asarray(inputs["la1"], dtype=np.int64)
    la2 = np.asarray(inputs["la2"], dtype=np.int64)
    shifted = np.asarray(inputs["shifted"], dtype=np.int64)
    union_idx = np.asarray(inputs["union_idx"], dtype=np.int64)

    if xpsi.shape != (J, B, C, M, N):
        return _fallback(xpsi, masks_shift, la1, la2, shifted, union_idx)
    prep = _host_prep(la1, la2, shifted, union_idx, masks_shift)
    if prep is None:
        return _fallback(xpsi, masks_shift, la1, la2, shifted, union_idx)
    try:
        return _run_device(xpsi, masks_shift, shifted, union_idx, prep)
    except Exception:
        return _fallback(xpsi, masks_shift, la1, la2, shifted, union_idx)


def _run_device(xpsi, masks_shift, shifted, union_idx, prep):
    if "prog" not in _CACHE:
        _CACHE["prog"] = _build_program(prep)
    nc = _CACHE["prog"]
    cst = _consts(prep, masks_shift, shifted)
    in_maps = _make_in_maps(xpsi, prep, cst)

    from concourse.bass_utils import run_bass_kernel_spmd
    res = run_bass_kernel_spmd(nc, in_maps, list(range(NCORES)))

    out = np.empty((640, B, len(union_idx)), np.float32)
    ridx = prep["ridx"]
    n_rows = prep["n_rows"]
    for core in range(NCORES):
        b, p = divmod(core, 2)
        dev = res.results[core]["out"]              # [n_rows, NX, NY] x-major
        out[ridx[p], b, :] = dev.reshape(n_rows, NX * NY)
    return out


if __name__ == "__main__":
    import importlib
    ref = importlib.import_module("reference")
    import jax
    cpu = jax.devices("cpu")[0]
    with jax.default_device(cpu):
        raw = ref.setup_inputs()
        ins = {k: np.asarray(v) for k, v in raw.items()}
        exp = np.asarray(ref.reference(**{k: jax.device_put(v, cpu) for k, v in raw.items()}))
    got = kernel(**ins)
    d = np.linalg.norm(got - exp) / np.linalg.norm(exp)
    print("rel:", d, "maxabs:", np.abs(got - exp).max())
